# revision 9
# baseline (speedup 1.0000x reference)
"""Trainium2 Bass kernel for nn_CRM_57861799411851 (ragged_sequence).

Pipeline (reference semantics):
  backproject(depth,K_inv) -> surface normals -> left/right side regions from
  mask edges -> inlier selection vs region-mean normal (cos > 0.2) -> ragged
  evenly-spaced sampling of S=2000 inliers -> per-row mean sample column ->
  row-wise linear depth interpolation inside the mask.

Device strategy (8 cores, row-sharded 192 rows/core):
  Launch A: dense mask edge scan -> per-row first-edge cols; indirect-DMA
    gather of narrow column bands (depth 3x66, mask 108) around the edges;
    normals/cos/inlier computed only on bands; AllReduce of region normal
    sums; inlier bitmaps out.
  Host glue: exact f32 replication of the ragged sampling on the tiny
    bitmaps (O(S+H)).
  Launch B: full-image row interpolation + mask blend (memory bound).
"""

import os
import numpy as np

H, W = 1536, 2048
S = 2000
SIDE = 20
THR = 0.2
NCORES = 8
R = H // NCORES            # 192 rows per core
BW = 64                    # band width (normals computed on band cols)
GW = BW + 2                # gathered depth cols (halo +-1)
MW = 108                   # gathered mask window cols
C0_LO, C0_HI = 20, 1960    # clamp for band start c0d (c0m = c0d-20, +108 <= 2048)

_CACHE = {}


def _build_a():
    import concourse.bass as bass
    import concourse.tile as tile
    from concourse import bacc, mybir
    import bass_rust

    dt = mybir.dt
    Alu = mybir.AluOpType
    AX = mybir.AxisListType

    nc = bacc.Bacc()
    depth_in = nc.declare_dram_parameter("depth_sh", [R + 2, W], dt.float32, isOutput=False)
    mask_in = nc.declare_dram_parameter("mask_sh", [R, W], dt.int32, isOutput=False)
    yv_in = nc.declare_dram_parameter("yv", [R, 2], dt.float32, isOutput=False)
    kinv_in = nc.declare_dram_parameter("kinv", [3, 3], dt.float32, isOutput=False)

    edges_out = nc.declare_dram_parameter("edges", [R, 2], dt.float32, isOutput=True)
    maskany_out = nc.declare_dram_parameter("maskany", [R, 1], dt.float32, isOutput=True)
    inlL_out = nc.declare_dram_parameter("inlL", [R, BW], dt.int8, isOutput=True)
    inlR_out = nc.declare_dram_parameter("inlR", [R, BW], dt.int8, isOutput=True)
    regL_out = nc.declare_dram_parameter("regL", [R, BW], dt.int8, isOutput=True)
    regR_out = nc.declare_dram_parameter("regR", [R, BW], dt.int8, isOutput=True)
    stats_out = nc.declare_dram_parameter("stats", [1, 24], dt.float32, isOutput=True)

    def cap(t, ap_dims, off=0):
        """Custom (possibly overlapping) free-dim view of a 2D sbuf tile."""
        a = t[:]
        return bass.AP(a.tensor, a.offset + off, [a.ap[0]] + ap_dims)

    with tile.TileContext(nc) as tc:
        with (
            tc.tile_pool(name="const", bufs=1) as cpool,
            tc.tile_pool(name="dense", bufs=2) as dpool,
            tc.tile_pool(name="small", bufs=2) as spool,
            tc.tile_pool(name="band", bufs=2) as bpool,
            tc.tile_pool(name="keep", bufs=3) as kpool,
            tc.tile_pool(name="acc", bufs=1) as apool,
            tc.tile_pool(name="dram", bufs=1, space="DRAM") as drpool,
        ):
            v, sc, gp = nc.vector, nc.scalar, nc.gpsimd

            # ---- constants ----
            iota_m16 = cpool.tile([128, W], dt.int16)   # x+1-4096
            gp.iota(iota_m16[:], [[1, W]], base=1 - 4096, channel_multiplier=0)
            iota66i = cpool.tile([128, GW], dt.int32)
            gp.iota(iota66i[:], [[1, GW]], base=0, channel_multiplier=0)
            iota66f = cpool.tile([128, GW], dt.float32)
            v.tensor_copy(iota66f[:], iota66i[:])
            eps6 = cpool.tile([128, 1], dt.float32)
            v.memset(eps6[:], 1e-6)
            k1 = cpool.tile([1, 9], dt.float32)
            gp.dma_start(k1[:], kinv_in[:].rearrange("a b -> () (a b)"))
            kbc = cpool.tile([128, 9], dt.float32)
            gp.partition_broadcast(kbc[:], k1[:])

            fc_scr = drpool.tile([R, 2], dt.float32)

            # ================= dense stage: first edge col per row/side ====
            for p0, P in ((0, 128), (128, 64)):
                m32 = dpool.tile([128, W], dt.int32, tag="m32")
                gp.dma_start(m32[:P, :], mask_in[p0:p0 + P, :])
                e8 = dpool.tile([128, W], dt.int8, tag="e8")
                v.tensor_tensor(e8[:P, 0:W - 1], m32[:P, 1:W], m32[:P, 0:W - 1],
                                op=Alu.subtract)
                v.tensor_scalar(e8[:P, W - 1:W], m32[:P, W - 1:W], -1.0, None,
                                op0=Alu.mult)
                ma = spool.tile([128, 1], dt.int32, tag="ma")
                v.tensor_reduce(ma[:P, :], m32[:P, :], axis=AX.X, op=Alu.max)
                maf = spool.tile([128, 1], dt.float32, tag="maf")
                v.tensor_copy(maf[:P, :], ma[:P, :])
                gp.dma_start(maskany_out[p0:p0 + P, :], maf[:P, :])
                for col, cmp in ((0, 1.0), (1, -1.0)):
                    v16 = dpool.tile([128, W], dt.int16, tag="v16")
                    v.scalar_tensor_tensor(v16[:P, :], e8[:P, :], cmp,
                                           iota_m16[:P, :], op0=Alu.is_equal,
                                           op1=Alu.mult)
                    m1 = spool.tile([128, 1], dt.int16, tag="m1")
                    v.tensor_reduce(m1[:P, :], v16[:P, :], axis=AX.X, op=Alu.min)
                    fcf = spool.tile([128, 1], dt.float32, tag="fcf")
                    v.tensor_scalar(fcf[:P, :], m1[:P, :], 4095.0, None, op0=Alu.add)
                    gp.dma_start(fc_scr[p0:p0 + P, col:col + 1], fcf[:P, :])
                    gp.dma_start(edges_out[p0:p0 + P, col:col + 1], fcf[:P, :])

            # ================= band stage =================================
            # band tiles: BT0 = L rows 0:128; BT1 = L rows 128:192 (p 0:64) +
            # R rows 0:64 (p 64:128); BT2 = R rows 64:192.
            BT_RUNS = [
                [(0, 128, 0, 0)],
                [(0, 64, 0, 128), (64, 128, 1, 0)],
                [(0, 128, 1, 64)],
            ]
            acc = apool.tile([128, 8], dt.float32)
            v.memset(acc[:], 0.0)

            keep_nf, keep_nn, keep_reg8 = [], [], []

            for bt, runs in enumerate(BT_RUNS):
                asm = bpool.tile([128, 3], dt.float32, tag="asm")
                sidecmp = bpool.tile([128, 1], dt.float32, tag="sidecmp")
                for (q0, q1, side, row0) in runs:
                    n = q1 - q0
                    gp.dma_start(asm[q0:q1, 0:2], yv_in[row0:row0 + n, 0:2])
                    gp.dma_start(asm[q0:q1, 2:3], fc_scr[row0:row0 + n, side:side + 1])
                    v.memset(sidecmp[q0:q1, :], 1.0 if side == 0 else -1.0)
                yab = asm[:, 0:1]
                yloc = asm[:, 1:2]
                fc = asm[:, 2:3]

                c0d = bpool.tile([128, 1], dt.float32, tag="c0d")
                v.tensor_scalar(c0d[:], fc, -21.0, None, op0=Alu.add)
                v.tensor_scalar(c0d[:], c0d[:], float(C0_LO), float(C0_HI),
                                op0=Alu.max, op1=Alu.min)
                hasE = bpool.tile([128, 1], dt.float32, tag="hasE")
                v.tensor_scalar(hasE[:], fc, 4000.0, None, op0=Alu.is_lt)

                offs = bpool.tile([128, 4], dt.float32, tag="offs")
                # col0: mask offset (yloc*W + c0d-20); col1..3: depth dy=0..2
                v.tensor_scalar(offs[:, 1:2], yloc, float(W), c0d[:], op0=Alu.mult,
                                op1=Alu.add)
                v.tensor_scalar(offs[:, 0:1], offs[:, 1:2], -20.0, None, op0=Alu.add)
                v.tensor_scalar(offs[:, 2:3], offs[:, 1:2], float(W), None, op0=Alu.add)
                v.tensor_scalar(offs[:, 3:4], offs[:, 1:2], float(2 * W), None,
                                op0=Alu.add)
                offi = bpool.tile([128, 4], dt.int32, tag="offi")
                v.tensor_copy(offi[:], offs[:])

                m32b = bpool.tile([128, MW], dt.int32, tag="m32b")
                gp.indirect_dma_start(
                    m32b[:], None, mask_in[:],
                    bass.IndirectOffsetOnAxis(ap=offi[:, 0:1], axis=1))
                g3 = bpool.tile([128, 3 * GW], dt.float32, tag="g3")
                for dy in range(3):
                    gp.indirect_dma_start(
                        g3[:, dy * GW:(dy + 1) * GW], None, depth_in[:],
                        bass.IndirectOffsetOnAxis(ap=offi[:, 1 + dy:2 + dy], axis=1))

                # ---- region recompute on the mask window (int8) ----
                m8 = bpool.tile([128, MW], dt.int8, tag="m8")
                v.tensor_scalar(m8[:], m32b[:], 0.0, None, op0=Alu.not_equal)
                e8b = bpool.tile([128, MW], dt.int8, tag="e8b")
                v.tensor_tensor(e8b[:, 0:MW - 1], m8[:, 1:MW], m8[:, 0:MW - 1],
                                op=Alu.subtract)
                eq = bpool.tile([128, MW], dt.int8, tag="eq")
                v.tensor_scalar(eq[:, 0:MW - 1], e8b[:, 0:MW - 1], sidecmp[:], None,
                                op0=Alu.is_equal)
                # dilation: forward OR window of 41 via log steps
                w1 = bpool.tile([128, MW], dt.int8, tag="w1")
                v.tensor_tensor(w1[:, 0:106], eq[:, 0:106], eq[:, 1:107], op=Alu.bitwise_or)
                w2 = bpool.tile([128, MW], dt.int8, tag="w2")
                v.tensor_tensor(w2[:, 0:104], w1[:, 0:104], w1[:, 2:106], op=Alu.bitwise_or)
                w3 = bpool.tile([128, MW], dt.int8, tag="w3")
                v.tensor_tensor(w3[:, 0:100], w2[:, 0:100], w2[:, 4:104], op=Alu.bitwise_or)
                w4 = bpool.tile([128, MW], dt.int8, tag="w4")
                v.tensor_tensor(w4[:, 0:92], w3[:, 0:92], w3[:, 8:100], op=Alu.bitwise_or)
                w5 = bpool.tile([128, MW], dt.int8, tag="w5")
                v.tensor_tensor(w5[:, 0:76], w4[:, 0:76], w4[:, 16:92], op=Alu.bitwise_or)
                w6 = bpool.tile([128, MW], dt.int8, tag="w6")
                v.tensor_tensor(w6[:, 0:68], w5[:, 0:68], w3[:, 32:100], op=Alu.bitwise_or)
                w7 = bpool.tile([128, MW], dt.int8, tag="w7")
                v.tensor_tensor(w7[:, 0:67], w6[:, 0:67], eq[:, 40:107], op=Alu.bitwise_or)
                # region = win41[j+1] & ~mask[21+j] & hasE, j in 0..63
                reg8 = kpool.tile([128, BW], dt.int8, tag="reg8")
                notm = bpool.tile([128, BW], dt.int8, tag="notm")
                v.tensor_scalar(notm[:], m8[:, 21:21 + BW], 0.0, None, op0=Alu.is_equal)
                v.tensor_tensor(reg8[:], w7[:, 1:1 + BW], notm[:], op=Alu.mult)
                v.tensor_scalar(reg8[:], reg8[:], hasE[:], None, op0=Alu.mult)
                regf = bpool.tile([128, BW], dt.float32, tag="regf")
                v.tensor_copy(regf[:], reg8[:])

                # ---- cam points (3 comps x 3 rows x 66 cols) ----
                uabs = bpool.tile([128, GW], dt.float32, tag="uabs")
                v.tensor_scalar(uabs[:], iota66f[:], c0d[:], None, op0=Alu.add)
                ya = bpool.tile([128, 3], dt.float32, tag="ya")
                for dy in range(3):
                    v.tensor_scalar(ya[:, dy:dy + 1], yab, float(dy - 1), None,
                                    op0=Alu.add)
                cams = []
                for ci in range(3):
                    ki1y = bpool.tile([128, 3], dt.float32, tag=f"ki1y{ci}")
                    v.tensor_tensor(ki1y[:], ya[:],
                                    kbc[:, 3 * ci + 1:3 * ci + 2].to_broadcast([128, 3]),
                                    op=Alu.mult)
                    t1 = bpool.tile([128, GW], dt.float32, tag=f"t1c{ci}")
                    v.tensor_scalar(t1[:], uabs[:], kbc[:, 3 * ci:3 * ci + 1], None,
                                    op0=Alu.mult)
                    a_i = bpool.tile([128, 3 * GW], dt.float32, tag=f"ai{ci}")
                    for dy in range(3):
                        v.tensor_scalar(a_i[:, dy * GW:(dy + 1) * GW], t1[:],
                                        ki1y[:, dy:dy + 1],
                                        kbc[:, 3 * ci + 2:3 * ci + 3],
                                        op0=Alu.add, op1=Alu.add)
                    cam = bpool.tile([128, 3 * GW], dt.float32, tag=f"cam{ci}")
                    v.tensor_tensor(cam[:], a_i[:], g3[:], op=Alu.mult)
                    cams.append(cam)

                # ---- neighbor diffs, 8 slots per comp ----
                # slots: 0 x0y0 1 y0 2 x1y0 (dy0) | 3 x0 4 x1 (dy1) | 5 x0y1 6 y1 7 x1y1 (dy2)
                Ds = []
                for ci in range(3):
                    cam = cams[ci]
                    D = bpool.tile([128, 8 * BW], dt.float32, tag=f"D{ci}")
                    ctr3 = cap(cam, [[0, 3], [1, BW]], off=GW + 1)
                    ctr2 = cap(cam, [[0, 2], [1, BW]], off=GW + 1)
                    v.tensor_tensor(cap(D, [[BW, 3], [1, BW]], off=0),
                                    cap(cam, [[1, 3], [1, BW]], off=0), ctr3,
                                    op=Alu.subtract)
                    v.tensor_tensor(cap(D, [[BW, 2], [1, BW]], off=3 * BW),
                                    cap(cam, [[2, 2], [1, BW]], off=GW), ctr2,
                                    op=Alu.subtract)
                    v.tensor_tensor(cap(D, [[BW, 3], [1, BW]], off=5 * BW),
                                    cap(cam, [[1, 3], [1, BW]], off=2 * GW), ctr3,
                                    op=Alu.subtract)
                    Ds.append(D)

                # ---- cross products: pairs (A x B) ----
                # G1: n0 = D3 x D1, n1 = D4 x D6 ; G2: n2 = D0 x D5, n3 = D2 x D7
                GA = [(3 * BW, BW), (0, 2 * BW)]      # (A off, A step)
                GB = [(BW, 5 * BW), (5 * BW, 2 * BW)]  # (B off, B step)
                crossp = bpool.tile([128, 3 * 4 * BW], dt.float32, tag="crossp")
                tmp1 = bpool.tile([128, 2 * BW], dt.float32, tag="tmp1")
                tmp2 = bpool.tile([128, 2 * BW], dt.float32, tag="tmp2")
                for c_out in range(3):
                    c1, c2 = (c_out + 1) % 3, (c_out + 2) % 3
                    for g in range(2):
                        ao, ast = GA[g]
                        bo, bst = GB[g]
                        A1 = cap(Ds[c1], [[ast, 2], [1, BW]], off=ao)
                        A2 = cap(Ds[c2], [[ast, 2], [1, BW]], off=ao)
                        B1 = cap(Ds[c1], [[bst, 2], [1, BW]], off=bo)
                        B2 = cap(Ds[c2], [[bst, 2], [1, BW]], off=bo)
                        v.tensor_tensor(tmp1[:], A1, B2, op=Alu.mult)
                        v.tensor_tensor(tmp2[:], A2, B1, op=Alu.mult)
                        v.tensor_tensor(
                            crossp[:, (c_out * 4 + g * 2) * BW:(c_out * 4 + g * 2 + 2) * BW],
                            tmp1[:], tmp2[:], op=Alu.subtract)

                # ---- normalize the 4 cross vectors (packed [128, 4*BW]) ----
                PK = 4 * BW
                sq = bpool.tile([128, PK], dt.float32, tag="sq")
                sqt = bpool.tile([128, PK], dt.float32, tag="sqt")
                v.tensor_tensor(sq[:], crossp[:, 0:PK], crossp[:, 0:PK], op=Alu.mult)
                v.tensor_tensor(sqt[:], crossp[:, PK:2 * PK], crossp[:, PK:2 * PK], op=Alu.mult)
                v.tensor_tensor(sq[:], sq[:], sqt[:], op=Alu.add)
                v.tensor_tensor(sqt[:], crossp[:, 2 * PK:3 * PK], crossp[:, 2 * PK:3 * PK], op=Alu.mult)
                v.tensor_tensor(sq[:], sq[:], sqt[:], op=Alu.add)
                den = bpool.tile([128, PK], dt.float32, tag="den")
                sc.sqrt(den[:], sq[:])
                sc.activation(den[:], den[:], mybir.ActivationFunctionType.Identity,
                              bias=eps6[:, 0:1])
                rec = bpool.tile([128, PK], dt.float32, tag="rec")
                v.reciprocal(rec[:], den[:])
                nrm = bpool.tile([128, 3 * PK], dt.float32, tag="nrm")
                for ci in range(3):
                    v.tensor_tensor(nrm[:, ci * PK:(ci + 1) * PK],
                                    crossp[:, ci * PK:(ci + 1) * PK], rec[:], op=Alu.mult)

                # ---- average of the 4 normals, * 0.25 ----
                s4 = bpool.tile([128, 3 * BW], dt.float32, tag="s4")
                for ci in range(3):
                    v.tensor_reduce(s4[:, ci * BW:(ci + 1) * BW],
                                    cap(nrm, [[1, BW], [BW, 4]], off=ci * PK),
                                    axis=AX.X, op=Alu.add)
                v.tensor_scalar(s4[:], s4[:], 0.25, None, op0=Alu.mult)

                # ---- final normalize + border zero ----
                sq2 = bpool.tile([128, BW], dt.float32, tag="sq2")
                sqt2 = bpool.tile([128, BW], dt.float32, tag="sqt2")
                v.tensor_tensor(sq2[:], s4[:, 0:BW], s4[:, 0:BW], op=Alu.mult)
                v.tensor_tensor(sqt2[:], s4[:, BW:2 * BW], s4[:, BW:2 * BW], op=Alu.mult)
                v.tensor_tensor(sq2[:], sq2[:], sqt2[:], op=Alu.add)
                v.tensor_tensor(sqt2[:], s4[:, 2 * BW:3 * BW], s4[:, 2 * BW:3 * BW], op=Alu.mult)
                v.tensor_tensor(sq2[:], sq2[:], sqt2[:], op=Alu.add)
                den2 = bpool.tile([128, BW], dt.float32, tag="den2")
                sc.sqrt(den2[:], sq2[:])
                sc.activation(den2[:], den2[:], mybir.ActivationFunctionType.Identity,
                              bias=eps6[:, 0:1])
                rec2 = bpool.tile([128, BW], dt.float32, tag="rec2")
                v.reciprocal(rec2[:], den2[:])
                # border mask folded into rec2
                bmask = bpool.tile([128, BW], dt.float32, tag="bmask")
                bm2 = bpool.tile([128, BW], dt.float32, tag="bm2")
                v.tensor_scalar(bmask[:], uabs[:, 1:1 + BW], 0.0, None, op0=Alu.is_gt)
                v.tensor_scalar(bm2[:], uabs[:, 1:1 + BW], float(W - 1), None, op0=Alu.is_lt)
                v.tensor_tensor(bmask[:], bmask[:], bm2[:], op=Alu.mult)
                byy = bpool.tile([128, 1], dt.float32, tag="byy")
                by2 = bpool.tile([128, 1], dt.float32, tag="by2")
                v.tensor_scalar(byy[:], yab, 0.5, None, op0=Alu.is_gt)
                v.tensor_scalar(by2[:], yab, float(H) - 1.5, None, op0=Alu.is_lt)
                v.tensor_tensor(byy[:], byy[:], by2[:], op=Alu.mult)
                v.tensor_scalar(bmask[:], bmask[:], byy[:], None, op0=Alu.mult)
                v.tensor_tensor(rec2[:], rec2[:], bmask[:], op=Alu.mult)
                nf = kpool.tile([128, 3 * BW], dt.float32, tag="nf")
                for ci in range(3):
                    v.tensor_tensor(nf[:, ci * BW:(ci + 1) * BW],
                                    s4[:, ci * BW:(ci + 1) * BW], rec2[:], op=Alu.mult)

                # ---- nn = |nf| ----
                v.tensor_tensor(sq2[:], nf[:, 0:BW], nf[:, 0:BW], op=Alu.mult)
                v.tensor_tensor(sqt2[:], nf[:, BW:2 * BW], nf[:, BW:2 * BW], op=Alu.mult)
                v.tensor_tensor(sq2[:], sq2[:], sqt2[:], op=Alu.add)
                v.tensor_tensor(sqt2[:], nf[:, 2 * BW:3 * BW], nf[:, 2 * BW:3 * BW], op=Alu.mult)
                v.tensor_tensor(sq2[:], sq2[:], sqt2[:], op=Alu.add)
                nn = kpool.tile([128, BW], dt.float32, tag="nn")
                sc.sqrt(nn[:], sq2[:])

                # ---- region partial sums into acc ----
                rn = bpool.tile([128, 3 * BW], dt.float32, tag="rn")
                for ci in range(3):
                    v.tensor_tensor(rn[:, ci * BW:(ci + 1) * BW],
                                    nf[:, ci * BW:(ci + 1) * BW], regf[:], op=Alu.mult)
                red = bpool.tile([128, 4], dt.float32, tag="red")
                for ci in range(3):
                    v.tensor_reduce(red[:, ci:ci + 1], rn[:, ci * BW:(ci + 1) * BW],
                                    axis=AX.X, op=Alu.add)
                v.tensor_reduce(red[:, 3:4], regf[:], axis=AX.X, op=Alu.add)
                for (q0, q1, side, row0) in runs:
                    base = 4 * side
                    v.tensor_tensor(acc[q0:q1, base:base + 4], acc[q0:q1, base:base + 4],
                                    red[q0:q1, 0:4], op=Alu.add)

                keep_nf.append(nf)
                keep_nn.append(nn)
                keep_reg8.append(reg8)

            # ---- cross-partition + cross-core reduce of sums ----
            accr = apool.tile([128, 8], dt.float32)
            gp.partition_all_reduce(accr[:], acc[:], 128, bass_rust.ReduceOp.add)
            cc_in = drpool.tile([1, 8], dt.float32)
            cc_out = drpool.tile([1, 8], dt.float32)
            gp.dma_start(cc_in[:], accr[0:1, :])
            gp.collective_compute(
                "AllReduce", mybir.AluOpType.add,
                replica_groups=[list(range(NCORES))],
                ins=[cc_in[:]], outs=[cc_out[:]])
            allr1 = apool.tile([1, 8], dt.float32)
            gp.dma_start(allr1[:], cc_out[:])
            allr = apool.tile([128, 8], dt.float32)
            gp.partition_broadcast(allr[:], allr1[:])

            # ---- mean normal per side: m4[side] = (mx,my,mz,mden) ----
            m4s = []
            stats_dbg = apool.tile([128, 24], dt.float32)
            v.memset(stats_dbg[:], 0.0)
            v.tensor_copy(stats_dbg[:, 0:8], allr[:])
            for side in range(2):
                b = 4 * side
                mc = apool.tile([128, 1], dt.float32, tag=f"mc{side}")
                v.tensor_scalar(mc[:], allr[:, b + 3:b + 4], 1.0, None, op0=Alu.max)
                mcr = apool.tile([128, 1], dt.float32, tag=f"mcr{side}")
                v.reciprocal(mcr[:], mc[:])
                mraw = apool.tile([128, 3], dt.float32, tag=f"mraw{side}")
                v.tensor_scalar(mraw[:], allr[:, b:b + 3], mcr[:], None, op0=Alu.mult)
                nsq = apool.tile([128, 1], dt.float32, tag=f"nsq{side}")
                nt = apool.tile([128, 1], dt.float32, tag=f"nt{side}")
                v.tensor_tensor(nsq[:], mraw[:, 0:1], mraw[:, 0:1], op=Alu.mult)
                v.tensor_tensor(nt[:], mraw[:, 1:2], mraw[:, 1:2], op=Alu.mult)
                v.tensor_tensor(nsq[:], nsq[:], nt[:], op=Alu.add)
                v.tensor_tensor(nt[:], mraw[:, 2:3], mraw[:, 2:3], op=Alu.mult)
                v.tensor_tensor(nsq[:], nsq[:], nt[:], op=Alu.add)
                mnorm = apool.tile([128, 1], dt.float32, tag=f"mnorm{side}")
                sc.sqrt(mnorm[:], nsq[:])
                v.tensor_scalar(mnorm[:], mnorm[:], 1e-12, None, op0=Alu.max)
                mnr = apool.tile([128, 1], dt.float32, tag=f"mnr{side}")
                v.reciprocal(mnr[:], mnorm[:])
                m4 = apool.tile([128, 4], dt.float32, tag=f"m4{side}")
                v.tensor_scalar(m4[:, 0:3], mraw[:], mnr[:], None, op0=Alu.mult)
                # mden = max(|mean_n|, 1e-8)
                v.tensor_tensor(nsq[:], m4[:, 0:1], m4[:, 0:1], op=Alu.mult)
                v.tensor_tensor(nt[:], m4[:, 1:2], m4[:, 1:2], op=Alu.mult)
                v.tensor_tensor(nsq[:], nsq[:], nt[:], op=Alu.add)
                v.tensor_tensor(nt[:], m4[:, 2:3], m4[:, 2:3], op=Alu.mult)
                v.tensor_tensor(nsq[:], nsq[:], nt[:], op=Alu.add)
                sc.sqrt(nt[:], nsq[:])
                v.tensor_scalar(m4[:, 3:4], nt[:], 1e-8, None, op0=Alu.max)
                v.tensor_copy(stats_dbg[:, 8 + 4 * side:8 + 4 * side + 4], m4[:])
                m4s.append(m4)

            gp.dma_start(stats_out[:], stats_dbg[0:1, 0:24])

            # ---- cos + inlier per band tile ----
            INL_DSTS = [
                [(0, 128, inlL_out, regL_out, 0)],
                [(0, 64, inlL_out, regL_out, 128), (64, 128, inlR_out, regR_out, 0)],
                [(0, 128, inlR_out, regR_out, 64)],
            ]
            for bt, runs in enumerate(BT_RUNS):
                nf, nn, reg8 = keep_nf[bt], keep_nn[bt], keep_reg8[bt]
                msel = bpool.tile([128, 4], dt.float32, tag="msel")
                isl2 = bpool.tile([128, 4], dt.int8, tag="isl2")
                for (q0, q1, side, row0) in runs:
                    v.memset(isl2[q0:q1, :], 1 if side == 0 else 0)
                v.select(msel[:], isl2[:], m4s[0][:], m4s[1][:])
                num = bpool.tile([128, 3 * BW], dt.float32, tag="num")
                v.tensor_tensor(
                    cap(num, [[BW, 3], [1, BW]]),
                    cap(nf, [[BW, 3], [1, BW]]),
                    bass.AP(msel[:].tensor, msel[:].offset,
                            [msel[:].ap[0], [1, 3], [0, BW]]),
                    op=Alu.mult)
                nsum = bpool.tile([128, BW], dt.float32, tag="nsum")
                v.tensor_reduce(nsum[:], cap(num, [[1, BW], [BW, 3]], off=0),
                                axis=AX.X, op=Alu.add)
                dn = bpool.tile([128, BW], dt.float32, tag="dn")
                v.tensor_scalar(dn[:], nn[:], 1e-8, None, op0=Alu.max)
                v.tensor_scalar(dn[:], dn[:], msel[:, 3:4], None, op0=Alu.mult)
                dr = bpool.tile([128, BW], dt.float32, tag="dr")
                v.reciprocal(dr[:], dn[:])
                cosv = bpool.tile([128, BW], dt.float32, tag="cosv")
                v.tensor_tensor(cosv[:], nsum[:], dr[:], op=Alu.mult)
                inl = bpool.tile([128, BW], dt.int8, tag="inl")
                v.tensor_scalar(inl[:], cosv[:], THR, None, op0=Alu.is_gt)
                v.tensor_tensor(inl[:], inl[:], reg8[:], op=Alu.mult)
                for (q0, q1, inl_dst, reg_dst, row0) in INL_DSTS[bt]:
                    n = q1 - q0
                    gp.dma_start(inl_dst[row0:row0 + n, :], inl[q0:q1, :])
                    gp.dma_start(reg_dst[row0:row0 + n, :], reg8[q0:q1, :])

    nc.finalize()
    return nc


def _build_b():
    import concourse.bass as bass
    import concourse.tile as tile
    from concourse import bacc, mybir

    dt = mybir.dt
    Alu = mybir.AluOpType

    nc = bacc.Bacc()
    depth_in = nc.declare_dram_parameter("depth_sh", [R, W], dt.float32, isOutput=False)
    mask_in = nc.declare_dram_parameter("mask_sh", [R, W], dt.int32, isOutput=False)
    rowp_in = nc.declare_dram_parameter("rowp", [R, 8], dt.float32, isOutput=False)
    out_b = nc.declare_dram_parameter("outb", [R, W], dt.float32, isOutput=True)

    with tile.TileContext(nc) as tc:
        with (
            tc.tile_pool(name="const", bufs=1) as cpool,
            tc.tile_pool(name="work", bufs=3) as wpool,
        ):
            v, gp = nc.vector, nc.gpsimd
            xio_i = cpool.tile([128, W], dt.int32)
            gp.iota(xio_i[:], [[1, W]], base=0, channel_multiplier=0)
            xio = cpool.tile([128, W], dt.float32)
            v.tensor_copy(xio[:], xio_i[:])
            for p0, P in ((0, 128), (128, 64)):
                dtile = wpool.tile([128, W], dt.float32, tag="dtile")
                gp.dma_start(dtile[:P, :], depth_in[p0:p0 + P, :])
                mtile = wpool.tile([128, W], dt.int32, tag="mtile")
                gp.dma_start(mtile[:P, :], mask_in[p0:p0 + P, :])
                rp = wpool.tile([128, 8], dt.float32, tag="rp")
                gp.dma_start(rp[:P, :], rowp_in[p0:p0 + P, :])
                cond = wpool.tile([128, W], dt.int8, tag="cond")
                v.tensor_scalar(cond[:P, :], mtile[:P, :], 0.0, rp[:P, 4:5],
                                op0=Alu.not_equal, op1=Alu.mult)
                r0t = wpool.tile([128, W], dt.float32, tag="r0t")
                v.tensor_scalar(r0t[:P, :], xio[:P, :], rp[:P, 0:1], rp[:P, 1:2],
                                op0=Alu.subtract, op1=Alu.mult)
                v.tensor_scalar(r0t[:P, :], r0t[:P, :], 0.0, 1.0,
                                op0=Alu.max, op1=Alu.min)
                v.tensor_scalar(r0t[:P, :], r0t[:P, :], rp[:P, 3:4], rp[:P, 2:3],
                                op0=Alu.mult, op1=Alu.add)
                v.copy_predicated(dtile[:P, :], cond[:P, :], r0t[:P, :])
                gp.dma_start(out_b[p0:p0 + P, :], dtile[:P, :])

    nc.finalize()
    return nc


def _run_spmd(nc, in_maps):
    if os.environ.get("BK_SIM"):
        from concourse.bass_interp import MultiCoreSim
        sim = MultiCoreSim(nc, NCORES, num_workers=NCORES)
        for c, im in enumerate(in_maps):
            for k, a in im.items():
                np.asarray(sim.cores[c].tensor(k))[:] = a
        sim.simulate()
        out = []
        out_names = [
            a.memorylocations[0].name
            for a in nc.m.functions[0].allocations
            if getattr(a, "kind", None) == "ExternalOutput"
        ]
        for c in range(NCORES):
            out.append({n: np.array(sim.cores[c].tensor(n)) for n in out_names})
        print("  sim time:", sim.global_time, "ns")
        return out, int(sim.global_time)
    from concourse.bass_utils import run_bass_kernel_spmd
    trace = bool(os.environ.get("BK_TRACE"))
    res = run_bass_kernel_spmd(nc, in_maps, list(range(NCORES)), trace=trace)
    if trace:
        print("  exec_time_ns:", res.exec_time_ns,
              "mean:", res.mean_exec_time_ns,
              "trace:", res.instructions_and_trace[1] if res.instructions_and_trace else None)
    return res.results, res.exec_time_ns


def _fallback(depth, mask, K_inv):
    import jax
    import jax.numpy as jnp
    cpu = jax.devices("cpu")[0]
    with jax.default_device(cpu):
        d = jnp.asarray(depth)
        m = jnp.asarray(mask)
        k = jnp.asarray(K_inv)
        mask_b = m.astype(bool)
        u = jnp.arange(W, dtype=jnp.float32)
        vv_ = jnp.arange(H, dtype=jnp.float32)
        uu, vv = jnp.meshgrid(u, vv_, indexing="xy")
        pix = jnp.stack([uu, vv, jnp.ones_like(uu)], 0).reshape(3, -1)
        cam = (k @ pix) * d.reshape(-1)[None, :]
        cp = cam.T.reshape(H, W, 3)

        def _norm(x, eps=1e-6):
            return x / (jnp.linalg.norm(x, axis=-1, keepdims=True) + eps)

        ctr = cp[1:-1, 1:-1]
        x0, x1 = cp[1:-1, :-2], cp[1:-1, 2:]
        y0, y1 = cp[:-2, 1:-1], cp[2:, 1:-1]
        x0y0, x0y1 = cp[:-2, :-2], cp[2:, :-2]
        x1y0, x1y1 = cp[:-2, 2:], cp[2:, 2:]
        n0 = _norm(jnp.cross(x0 - ctr, y0 - ctr))
        n1 = _norm(jnp.cross(x1 - ctr, y1 - ctr))
        n2 = _norm(jnp.cross(x0y0 - ctr, x0y1 - ctr))
        n3 = _norm(jnp.cross(x1y0 - ctr, x1y1 - ctr))
        n = _norm((n0 + n1 + n2 + n3) / 4.0)
        normals = jnp.zeros_like(cp).at[1:-1, 1:-1].set(n)

        mask_f = mask_b.astype(jnp.float32)
        grad_x = jnp.pad(mask_f, ((0, 0), (0, 1)))[:, 1:] - mask_f
        left_edge, right_edge = grad_x == 1, grad_x == -1

        def dil(edge):
            r = jax.lax.reduce_window(edge.astype(jnp.float32), 0.0, jax.lax.max,
                                      (1, 2 * SIDE + 1), (1, 1), [(0, 0), (SIDE, SIDE)])
            return r > 0

        not_mask = ~mask_b
        lreg = dil(left_edge) & not_mask
        rreg = dil(right_edge) & not_mask

        def inlier_coords(region):
            cnt = region.sum()
            mean = jnp.sum(jnp.where(region[..., None], normals, 0.0), axis=(0, 1)) / jnp.maximum(cnt, 1)
            mean = mean / jnp.maximum(jnp.linalg.norm(mean), 1e-12)
            nn_ = jnp.linalg.norm(normals, axis=-1)
            cos = (normals @ mean) / (jnp.maximum(nn_, 1e-8) * jnp.maximum(jnp.linalg.norm(mean), 1e-8))
            inl = region & (cos > THR)
            N = inl.sum()
            flat = jnp.where(inl.reshape(-1), jnp.arange(H * W), H * W)
            srt = jnp.sort(flat)
            i = jnp.arange(S)
            pos_big = jnp.floor(i.astype(jnp.float32) * (N - 1).astype(jnp.float32) / (S - 1)).astype(jnp.int32)
            pos_small = jnp.minimum(i, jnp.maximum(N - 1, 0))
            pos = jnp.where(N >= S, pos_big, pos_small)
            sel = srt[jnp.clip(pos, 0, H * W - 1)]
            sel = jnp.clip(sel, 0, H * W - 1)
            return sel // W, sel % W, N

        ly, lxc, Nl = inlier_coords(lreg)
        ry, rxc, Nr = inlier_coords(rreg)
        valid_cluster = (Nl > 0) & (Nr > 0)
        ones = jnp.ones((S,), jnp.float32)
        ls = jax.ops.segment_sum(lxc.astype(jnp.float32), ly, num_segments=H)
        lcc = jax.ops.segment_sum(ones, ly, num_segments=H)
        rs = jax.ops.segment_sum(rxc.astype(jnp.float32), ry, num_segments=H)
        rcc = jax.ops.segment_sum(ones, ry, num_segments=H)
        valid = (lcc > 0) & (rcc > 0) & valid_cluster
        lmean = ls / jnp.maximum(lcc, 1.0)
        rmean = rs / jnp.maximum(rcc, 1.0)
        lx = jnp.clip(jnp.round(lmean).astype(jnp.int32), 0, W - 1)
        rx = jnp.clip(jnp.round(rmean).astype(jnp.int32), 0, W - 1)
        ld = jnp.take_along_axis(d, lx[:, None], axis=1)[:, 0]
        rd = jnp.take_along_axis(d, rx[:, None], axis=1)[:, 0]
        valid = valid & ~jnp.isnan(ld) & ~jnp.isnan(rd) & (mask_b.sum(axis=1) > 0)
        xg = jnp.arange(W, dtype=jnp.float32)[None, :]
        lxf, rxf = lx.astype(jnp.float32)[:, None], rx.astype(jnp.float32)[:, None]
        ratios = jnp.clip((xg - lxf) / (rxf - lxf + 1e-6), 0.0, 1.0)
        interp = (1.0 - ratios) * ld[:, None] + ratios * rd[:, None]
        return np.array(jnp.where(mask_b & valid[:, None], interp, d))


def _edges_np(mask_b):
    g = np.zeros((H, W), np.int8)
    mb = mask_b.astype(np.int8)
    g[:, :W - 1] = mb[:, 1:] - mb[:, :W - 1]
    g[:, W - 1] = -mb[:, W - 1]
    return g


def _bands_ok(g, cmp):
    e = g == cmp
    has = e.any(1)
    if not has.any():
        return True
    fc = np.argmax(e, 1)
    lc = W - 1 - np.argmax(e[:, ::-1], 1)
    fc, lc = fc[has], lc[has]
    return bool((lc - fc <= 22).all() and (fc >= 41).all() and (fc <= 1981).all())


def kernel(depth, mask, K_inv):
    depth = np.ascontiguousarray(np.asarray(depth, np.float32))
    mask = np.ascontiguousarray(np.asarray(mask, np.int32))
    K = np.ascontiguousarray(np.asarray(K_inv, np.float32))

    g = _edges_np(mask != 0)
    if not (_bands_ok(g, 1) and _bands_ok(g, -1)):
        return _fallback(depth, mask, K)

    if "A" not in _CACHE:
        _CACHE["A"] = _build_a()
    in_maps = []
    for c in range(NCORES):
        r0 = c * R
        rows = np.clip(np.arange(r0 - 1, r0 + R + 1), 0, H - 1)
        yv = np.stack([np.arange(r0, r0 + R, dtype=np.float32),
                       np.arange(R, dtype=np.float32)], axis=1)
        in_maps.append({
            "depth_sh": depth[rows],
            "mask_sh": mask[r0:r0 + R],
            "yv": np.ascontiguousarray(yv),
            "kinv": K,
        })
    resA, tA = _run_spmd(_CACHE["A"], in_maps)

    edges = np.concatenate([r["edges"] for r in resA], 0)       # [H, 2]
    maskany = np.concatenate([r["maskany"] for r in resA], 0)[:, 0]
    inlL = np.concatenate([r["inlL"] for r in resA], 0)         # [H, BW]
    inlR = np.concatenate([r["inlR"] for r in resA], 0)

    def side_stats(bitmap, fc_col):
        c0 = np.clip(fc_col.astype(np.int64) - 21, C0_LO, C0_HI)
        ys, js = np.nonzero(bitmap)
        xs = c0[ys] + 1 + js
        flat = (ys * W + xs).astype(np.int64)
        N = int(flat.size)
        return flat, N

    flatL, Nl = side_stats(inlL, edges[:, 0])
    flatR, Nr = side_stats(inlR, edges[:, 1])

    def sample(flat, N):
        i = np.arange(S)
        if N >= S:
            pos = np.floor(i.astype(np.float32) * np.float32(N - 1)
                           / np.float32(S - 1)).astype(np.int32)
        else:
            pos = np.minimum(i, max(N - 1, 0)).astype(np.int32)
        pos = np.clip(pos, 0, H * W - 1)
        if N > 0:
            sel = np.where(pos < N, flat[np.minimum(pos, N - 1)], H * W)
        else:
            sel = np.full(S, H * W, np.int64)
        sel = np.clip(sel, 0, H * W - 1)
        return sel // W, sel % W

    ly, lxc = sample(flatL, Nl)
    ry, rxc = sample(flatR, Nr)

    def seg(yy, xx):
        s = np.zeros(H, np.float32)
        c = np.zeros(H, np.float32)
        np.add.at(s, yy, xx.astype(np.float32))
        np.add.at(c, yy, np.float32(1.0))
        return s, c

    ls, lcc = seg(ly, lxc)
    rs, rcc = seg(ry, rxc)
    lmean = ls / np.maximum(lcc, np.float32(1.0))
    rmean = rs / np.maximum(rcc, np.float32(1.0))
    lx = np.clip(np.round(lmean).astype(np.int32), 0, W - 1)
    rx = np.clip(np.round(rmean).astype(np.int32), 0, W - 1)
    ld = depth[np.arange(H), lx]
    rd = depth[np.arange(H), rx]
    valid = ((lcc > 0) & (rcc > 0) & (Nl > 0) & (Nr > 0)
             & ~np.isnan(ld) & ~np.isnan(rd) & (maskany > 0))

    den = (rx.astype(np.float32) - lx.astype(np.float32)) + np.float32(1e-6)
    inv = np.float32(1.0) / den
    rowp = np.zeros((H, 8), np.float32)
    rowp[:, 0] = lx.astype(np.float32)
    rowp[:, 1] = inv
    rowp[:, 2] = ld
    rowp[:, 3] = rd - ld
    rowp[:, 4] = valid.astype(np.float32)

    if "B" not in _CACHE:
        _CACHE["B"] = _build_b()
    in_maps_b = []
    for c in range(NCORES):
        r0 = c * R
        in_maps_b.append({
            "depth_sh": depth[r0:r0 + R],
            "mask_sh": mask[r0:r0 + R],
            "rowp": rowp[r0:r0 + R],
        })
    resB, tB = _run_spmd(_CACHE["B"], in_maps_b)
    out = np.concatenate([r["outb"] for r in resB], 0)

    kernel.last_exec_ns = ((tA or 0) + (tB or 0)) or None
    kernel.last_debug = {
        "edges": edges, "maskany": maskany, "inlL": inlL, "inlR": inlR,
        "regL": np.concatenate([r["regL"] for r in resA], 0),
        "regR": np.concatenate([r["regR"] for r in resA], 0),
        "stats": resA[0]["stats"], "Nl": Nl, "Nr": Nr,
        "lx": lx, "rx": rx, "valid": valid,
    }
    return out
